# revision 1
# baseline (speedup 1.0000x reference)
"""Trainium2 Bass kernel for nn_BinReg (histogram_binning dampening loss).

Computes: 0.1 * ( mean((wq - w)^2) + sum_k var_k ) where var_k is the
unbiased variance of w restricted to quant-bin k (16 bins), added only
when count_k > 1.

Strategy (8 NeuronCores, data-parallel over elements):
  - Shard the 4096x16384 tensors row-wise into 8 shards of [512, 16384],
    viewed as [128 partitions, 65536 free] per core.
  - Per core, stream tiles [128, FT]. Bin ids b = wq/alpha + 8 + 192 in
    bf16 land exactly on 192+k (bf16 ULP=1 in [128,256)), so is_equal
    masks are exact. Per tile, for bins 0..14:
      s_k  + masked w : one scalar_tensor_tensor (b==k)*w with fused
                        free-dim accumulate (DVE, also materializes mw)
      ss_k            : Square(mw) with fused accumulate (ACT engine,
                        runs in parallel with DVE)
      cnt_k           : single-src tensor_scalar (b==k) fused accumulate
                        (DVE 4x mode)
    plus per-tile totals sum(w) (DVE 2x) and sum(w^2) (ACT) so bin 15
    falls out by subtraction on the host; total count is positional.
  - MSE is reconstructed on the host from the bin stats:
      sum((wq-w)^2) = a^2*sum_k cnt_k q_k^2 - 2a*sum_k q_k s_k + sum_k ss_k
    (verified to 9e-9 relative against the direct sum).
  - Per-core partial sums land in SBUF accumulator columns (one column
    per (bin, tile)); DMA'd out and reduced on the host in float64.
"""

from functools import lru_cache

import numpy as np

import concourse.bacc as bacc
import concourse.bass as bass
import concourse.mybir as mybir
import concourse.tile as tile
from concourse.bass_utils import run_bass_kernel_spmd

P = 128
N_CORES = 8
ROWS, COLS = 4096, 16384
SHARD_ROWS = ROWS // N_CORES            # 512
FREE = SHARD_ROWS * COLS // P           # 65536 elements per partition
FT = 4096                               # tile free size
NBINS = 16
NB = NBINS - 1                          # bins computed on-device

F32 = mybir.dt.float32
BF16 = mybir.dt.bfloat16
ALU = mybir.AluOpType
ACTF = mybir.ActivationFunctionType
BMAG = 192.0  # bf16 magic base: b lands exactly on 192+k (ULP=1 in [128,256))

# Set by test.py; results stashed for inspection.
TRACE = False
LAST_RESULTS = None
REPEAT = 1  # timing aid: repeat the whole compute R times (same result)
GP_CNT = 0  # cnt passes offloaded to GPSIMD (unsupported by walrus on Pool)
GP_STT = 0  # s/mw STT passes on GPSIMD (unsupported by walrus on Pool)
SPLIT = False  # drain-aware split: measured slower than fused STT on HW
GP_TT = 7  # mw TT products offloaded to GPSIMD
RELU_CNT = True  # counts via ACT relu-accum differencing instead of DVE


@lru_cache(maxsize=4)
def _build(inv_alpha: float, free: int = FREE, ft: int = FT,
           repeat: int = 1, gp_cnt: int = 0, gp_stt: int = 0,
           split_mode: bool = False, gp_tt: int = 0,
           relu_cnt: bool = False) -> bass.Bass:
    NT = free // ft
    nc = bacc.Bacc(trn_type="TRN2")
    w_d = nc.dram_tensor("w", [P, free], F32, kind="ExternalInput")
    wq_d = nc.dram_tensor("wq", [P, free], F32, kind="ExternalInput")
    cnt_d = nc.dram_tensor("cnt", [P, NB * NT], F32, kind="ExternalOutput")
    s_d = nc.dram_tensor("s", [P, NB * NT], F32, kind="ExternalOutput")
    ss_d = nc.dram_tensor("ss", [P, NB * NT], F32, kind="ExternalOutput")
    tots_d = nc.dram_tensor("tots", [P, NT], F32, kind="ExternalOutput")
    totss_d = nc.dram_tensor("totss", [P, NT], F32, kind="ExternalOutput")
    rr_d = (
        nc.dram_tensor("rr", [P, NBINS * NT], F32, kind="ExternalOutput")
        if relu_cnt else None
    )

    with tile.TileContext(nc) as tc:
        with (
            tc.tile_pool(name="io", bufs=2) as io,
            tc.tile_pool(name="work", bufs=2) as work,
            tc.tile_pool(name="acc", bufs=1) as acc,
        ):
            cnt_a = acc.tile([P, NB * NT], F32, tag="cnt_a")
            s_a = acc.tile([P, NB * NT], F32, tag="s_a")
            ss_a = acc.tile([P, NB * NT], F32, tag="ss_a")
            tots_a = acc.tile([P, NT], F32, tag="tots_a")
            totss_a = acc.tile([P, NT], F32, tag="totss_a")
            rr_a = None
            bias_t = None
            if relu_cnt:
                rr_a = acc.tile([P, NBINS * NT], F32, tag="rr_a")
                bias_t = acc.tile([P, NBINS], F32, tag="bias_t")
                for t in range(NBINS):
                    nc.gpsimd.memset(bias_t[:, t : t + 1], -(BMAG + float(t)))

            import contextlib
            loop_cm = (
                tc.For_i(
                    0, repeat, 1,
                    hint_engines=(mybir.EngineType.DVE, mybir.EngineType.Activation),
                )
                if repeat > 1
                else contextlib.nullcontext()
            )
            with loop_cm:
                for i in range(NT):
                    sl = slice(i * ft, (i + 1) * ft)
                    w_t = io.tile([P, ft], F32, tag="w")
                    nc.sync.dma_start(w_t[:], w_d[:, sl])
                    wq_t = io.tile([P, ft], F32, tag="wq")
                    nc.sync.dma_start(wq_t[:], wq_d[:, sl])

                    # b = wq/alpha + 8 + 192, bf16-rounds exactly to 192+k.
                    # With relu_cnt the DVE only runs the 15 STT passes, so
                    # compute b on ACT via Copy's free affine (scale, bias).
                    b_bf = work.tile([P, ft], BF16, tag="b_bf")
                    if relu_cnt:
                        nc.scalar.activation(
                            b_bf[:], wq_t[:], ACTF.Copy,
                            bias=BMAG + 8.0, scale=inv_alpha,
                        )
                    else:
                        nc.vector.tensor_scalar(
                            b_bf[:], wq_t[:], inv_alpha, BMAG + 8.0,
                            op0=ALU.mult, op1=ALU.add,
                        )

                    # totals for bin-15-by-subtraction
                    if relu_cnt:
                        # total sum(w) on ACT (Copy func), freeing DVE
                        tc_t = work.tile([P, ft], BF16, tag="junk_act")
                        nc.scalar.activation(
                            tc_t[:], w_t[:], ACTF.Copy,
                            accum_out=tots_a[:, i : i + 1],
                        )
                    else:
                        tj_t = work.tile([P, ft], F32, tag="junk_dve32")
                        nc.vector.tensor_scalar(
                            tj_t[:], w_t[:], 1.0, None,
                            op0=ALU.mult, op1=ALU.add,
                            accum_out=tots_a[:, i : i + 1],
                        )
                    tsq_t = work.tile([P, ft], BF16, tag="junk_act")
                    nc.scalar.activation(
                        tsq_t[:], w_t[:], ACTF.Square,
                        accum_out=totss_a[:, i : i + 1],
                    )
                    if relu_cnt:
                        # R_t = sum relu(b - (192+t)) for t=0..15 on ACT.
                        # cnt_k = R_{k-1} - 2 R_k + R_{k+1} (exact ints),
                        # with R_{-1} = R_0 + n and R_16 = 0.
                        for t in range(NBINS):
                            rj_t = work.tile([P, ft], BF16, tag="junk_act")
                            nc.scalar.activation(
                                rj_t[:], b_bf[:], ACTF.Relu,
                                bias=bias_t[:, t : t + 1],
                                accum_out=rr_a[:, t * NT + i : t * NT + i + 1],
                            )

                    if split_mode:
                        # w in bf16 for the 2x-mode TT products
                        w_bf = work.tile([P, ft], BF16, tag="w_bf")
                        nc.vector.tensor_copy(w_bf[:], w_t[:])

                    for k in range(NB):
                        col = k * NT + i
                        if split_mode:
                            # mask + count in one 4x-mode tensor_scalar
                            m_t = work.tile([P, ft], BF16, tag="mask")
                            nc.vector.tensor_scalar(
                                m_t[:], b_bf[:], BMAG + float(k), None,
                                op0=ALU.is_equal, op1=ALU.add,
                                accum_out=cnt_a[:, col : col + 1],
                            )
                            # mw = mask * w (bf16 2x TT; split DVE/GPSIMD)
                            mw_t = work.tile([P, ft], BF16, tag="mw")
                            tt_eng = nc.gpsimd if k < gp_tt else nc.vector
                            tt_eng.tensor_mul(mw_t[:], m_t[:], w_bf[:])
                            # s_k: single-src 4x tensor_scalar accumulate
                            sj_t = work.tile([P, ft], BF16, tag="junk_dve")
                            nc.vector.tensor_scalar(
                                sj_t[:], mw_t[:], 1.0, None,
                                op0=ALU.mult, op1=ALU.add,
                                accum_out=s_a[:, col : col + 1],
                            )
                            # ss_k on ACT
                            sq_t = work.tile([P, ft], BF16, tag="junk_act")
                            nc.scalar.activation(
                                sq_t[:], mw_t[:], ACTF.Square,
                                accum_out=ss_a[:, col : col + 1],
                            )
                        else:
                            # masked w + fused sum -> s_k (one STT on DVE)
                            mw_t = work.tile([P, ft], F32, tag="mw")
                            nc.vector.scalar_tensor_tensor(
                                mw_t[:], b_bf[:], BMAG + float(k), w_t[:],
                                op0=ALU.is_equal, op1=ALU.mult,
                                accum_out=s_a[:, col : col + 1],
                            )
                            sq_t = work.tile([P, ft], BF16, tag="junk_act")
                            nc.scalar.activation(
                                sq_t[:], mw_t[:], ACTF.Square,
                                accum_out=ss_a[:, col : col + 1],
                            )
                            if not relu_cnt:
                                c_t = work.tile([P, ft], BF16, tag="junk_dve")
                                nc.vector.tensor_scalar(
                                    c_t[:], b_bf[:], BMAG + float(k), None,
                                    op0=ALU.is_equal, op1=ALU.add,
                                    accum_out=cnt_a[:, col : col + 1],
                                )

            if relu_cnt:
                nc.gpsimd.memset(cnt_a[:], 0.0)
                nc.sync.dma_start(rr_d[:], rr_a[:])
            nc.sync.dma_start(cnt_d[:], cnt_a[:])
            nc.sync.dma_start(s_d[:], s_a[:])
            nc.sync.dma_start(ss_d[:], ss_a[:])
            nc.sync.dma_start(tots_d[:], tots_a[:])
            nc.sync.dma_start(totss_d[:], totss_a[:])

    nc.finalize()
    return nc


def _reduce_stats(results, free):
    NT = free // FT
    cnt = np.zeros(NBINS, dtype=np.float64)
    s = np.zeros(NBINS, dtype=np.float64)
    ss = np.zeros(NBINS, dtype=np.float64)
    rr = np.zeros(NBINS, dtype=np.float64)
    have_rr = "rr" in results[0]
    n_total = 0.0
    for r in results:
        cnt[:NB] += r["cnt"].astype(np.float64).reshape(P, NB, NT).sum(axis=(0, 2))
        s[:NB] += r["s"].astype(np.float64).reshape(P, NB, NT).sum(axis=(0, 2))
        ss[:NB] += r["ss"].astype(np.float64).reshape(P, NB, NT).sum(axis=(0, 2))
        s[NB] += float(r["tots"].astype(np.float64).sum())
        ss[NB] += float(r["totss"].astype(np.float64).sum())
        if have_rr:
            rr += r["rr"].astype(np.float64).reshape(P, NBINS, NT).sum(axis=(0, 2))
        n_total += P * free
    if have_rr:
        # cnt_k = R_{k-1} - 2 R_k + R_{k+1}; R_{-1} = R_0 + n; R_16 = 0
        Rm = np.concatenate(([rr[0] + n_total], rr, [0.0]))  # R_{-1..16}
        cnt = np.round(Rm[:-2] - 2.0 * Rm[1:-1] + Rm[2:])
    else:
        cnt[NB] = n_total - cnt[:NB].sum()
    s[NB] -= s[:NB].sum()
    ss[NB] -= ss[:NB].sum()
    return cnt, s, ss, n_total


def kernel(weight, weight_q, nbit, alpha) -> np.ndarray:
    global LAST_RESULTS
    nb = int(np.asarray(nbit))
    qn = -(2 ** (nb - 1))
    qp = 2 ** (nb - 1) - 1
    nbins = qp - qn + 1
    assert nbins == NBINS, f"kernel hardcodes 16 bins, got {nbins}"
    a = float(np.asarray(alpha).reshape(-1)[0])

    w = np.ascontiguousarray(np.asarray(weight, dtype=np.float32)).reshape(
        N_CORES, P, FREE
    )
    wq = np.ascontiguousarray(np.asarray(weight_q, dtype=np.float32)).reshape(
        N_CORES, P, FREE
    )

    nc = _build(1.0 / a, FREE, FT, REPEAT, GP_CNT, GP_STT, SPLIT, GP_TT, RELU_CNT)
    in_maps = [{"w": w[i], "wq": wq[i]} for i in range(N_CORES)]
    res = run_bass_kernel_spmd(
        nc, in_maps, core_ids=list(range(N_CORES)), trace=TRACE
    )
    LAST_RESULTS = res

    cnt, s, ss, n_total = _reduce_stats(res.results, FREE)

    q = np.arange(NBINS, dtype=np.float64) + qn  # quant levels / alpha
    mse_sum = a * a * (cnt * q * q).sum() - 2.0 * a * (q * s).sum() + ss.sum()
    loss = mse_sum / n_total
    denom_n = np.maximum(cnt, 1.0)
    denom_nm1 = np.maximum(cnt - 1.0, 1.0)
    var = (ss - s * s / denom_n) / denom_nm1
    loss += float(np.where(cnt > 1.0, var, 0.0).sum())
    return np.asarray(0.1 * loss, dtype=np.float32)

